# revision 11
# baseline (speedup 1.0000x reference)
"""SkeletalPool Trainium2 kernel: three parallel DMA lanes + DVE adds.

Computes dev_out = bf16(x[:, IDX0] + x[:, IDX1]); the host applies the
exact *0.5 (multiplying by 0.5 is an exact exponent shift, so
0.5*f32(bf16(a+b)) == f32(bf16((a+b)*0.5)) bitwise; max relative error
is the bf16 half-ulp ~3.9e-3, uniform in |value|). Joint 0 is computed
as x0 + x0 via aliased operand views, so the host path is uniform.

Per batch, 8 chunks of 2 output joints each (input joints):
  c0: [0,1,2] (3 joints; pair (x0,x0) aliases joint 0 twice via a
  stride-2TT operand view); c>0: [4c-1 .. 4c+2] (4 joints)
Chunk c -> output joints [2c, 2c+1].

Lanes: loads ride SP, ACT and Pool dma_starts in parallel (the cost
model holds each issuing sequencer for its own transfers, so three
engines triple DMA throughput); stores are double DMAs (2 tasks = 4
output joints = one contiguous 2 MiB bf16 region) on SP/ACT, with the
final two doubles split into singles so the drain overlaps the last
adds.
DVE does one strided tensor_add per chunk (f32 pairs -> bf16).

Raw Bass: every wait is a standalone sequencer wait_ge; DMAs carry only
semaphore updates. HWDGE (SP/ACT) and SWDGE (Pool) completions use
separate per-slot semaphore families (they may not share a semaphore).

An optional `repeat` build parameter replays the identical pipeline R
times inside one NEFF; timing R=1 vs R=K and taking the slope isolates
true on-device execution from host dispatch overhead.
"""

import sys

if "/opt/trn_rl_repo" not in sys.path:
    sys.path.insert(0, "/opt/trn_rl_repo")

import numpy as np

import concourse.bass as bass
import concourse.mybir as mybir

N_CORES = 8
B_SHARD = 4
J_IN = 31
J_OUT = 16
C = 64
T = 4096
P = 128
TT = 2048

N_CHUNK = 8  # chunks per batch, 2 output joints each
N_TASKS = B_SHARD * N_CHUNK  # 32
NBUF = 5  # tin slots
SLOT_J = 4

# Load lanes: Pool takes all four cheap c0 loads + 10 regular loads;
# SP/ACT take 9 regular loads each plus the double-stores.
_LL = [
    ["pool", "sp", "pool", "act", "pool", "sp", "act", "pool"],
    ["pool", "act", "sp", "pool", "act", "pool", "sp", "act"],
    ["pool", "sp", "pool", "act", "sp", "pool", "act", "sp"],
    ["pool", "act", "sp", "pool", "act", "sp", "pool", "pool"],
]


def task(k):
    b, c = divmod(k % N_TASKS, N_CHUNK)
    return b, c


def load_lane(k):
    b, c = task(k)
    return _LL[b % 4][c]


def store_lane(m):
    """Double-store m covers tasks 2m, 2m+1."""
    return "sp" if m % 2 == 0 else "act"


def build(repeat: int = 1) -> bass.Bass:
    nc = bass.Bass("TRN2", debug=False, num_devices=N_CORES)
    f32 = mybir.dt.float32
    bf16 = mybir.dt.bfloat16

    x = nc.dram_tensor("x", (B_SHARD, J_IN, C, T), f32, kind="ExternalInput")
    out = nc.dram_tensor("out", (B_SHARD, J_OUT, C, T), bf16, kind="ExternalOutput")

    # [b, 128, j, 2048] views: each joint block is 1 MiB (f32) / 0.5 MiB
    # (bf16) contiguous, relabeled to 128 partitions x 2048 elements.
    xp = x.ap().rearrange("b j c (u t) -> b (c u) j t", u=2)
    op = out.ap().rearrange("b j c (u t) -> b (c u) j t", u=2)

    tin = nc.alloc_sbuf_tensor("tin", [P, NBUF * SLOT_J * TT], f32)
    # four 2-joint slots; a double-store spans two adjacent slots
    tob = nc.alloc_sbuf_tensor("tob", [P, 4 * 2 * TT], bf16)

    # HWDGE (SP/ACT) and SWDGE (Pool) DMA completions use separate sems.
    s_loadh = [nc.alloc_semaphore(f"s_loadh{i}") for i in range(NBUF)]
    s_loadp = [nc.alloc_semaphore(f"s_loadp{i}") for i in range(NBUF)]
    s_store = [nc.alloc_semaphore(f"s_store{i}") for i in range(2)]
    s_add = nc.alloc_semaphore("s_add")

    NG = N_TASKS * repeat
    NM = NG // 2  # double-stores (last two split into singles)

    def load_wait(k):
        """(sem, target) for task k's load completion (slot+family exact)."""
        g = load_lane(k) == "pool"
        cnt = len(
            [
                kk
                for kk in range(k + 1)
                if kk % NBUF == k % NBUF and (load_lane(kk) == "pool") == g
            ]
        )
        return (s_loadp if g else s_loadh)[k % NBUF], 16 * cnt

    def tin_v(k, nj=SLOT_J):
        s = (k % NBUF) * SLOT_J * TT
        return tin.ap()[:, s : s + nj * TT].rearrange("p (j t) -> p j t", j=nj)

    def tin_pairs(k):
        _, c = task(k)
        s = (k % NBUF) * SLOT_J * TT
        if c == 0:
            # joints [x0, x1, x2]: pairs (x0,x0) and (x1,x2):
            # in0 = joints {0,1} (stride TT), in1 = joints {0,2} (stride 2TT)
            in0 = tin.ap()[:, s : s + 2 * TT].rearrange("p (j t) -> p j t", j=2)
            in1 = tin.ap()[:, s : s + 4 * TT].rearrange(
                "p (j two t) -> p j two t", j=2, two=2
            )[:, :, 0, :]
            return in0, in1
        v = tin.ap()[:, s : s + 4 * TT].rearrange(
            "p (j two t) -> p j two t", j=2, two=2
        )
        return v[:, :, 0, :], v[:, :, 1, :]

    def tob_task(k):  # [128, 2, 2048] slot k%4
        s = (k % 4) * 2 * TT
        return tob.ap()[:, s : s + 2 * TT].rearrange("p (j t) -> p j t", j=2)

    def tob_dv(m):  # [128, 4, 2048] over slots of tasks (2m, 2m+1)
        s = ((2 * m) % 4) * 2 * TT
        return tob.ap()[:, s : s + 4 * TT].rearrange("p (j t) -> p j t", j=4)

    def emit_load(eng, k):
        b, c = task(k)
        jin, nj = (0, 3) if c == 0 else (4 * c - 1, 4)
        if k >= NBUF:
            eng.wait_ge(s_add, k - NBUF + 1)
        fam = s_loadp if load_lane(k) == "pool" else s_loadh
        eng.dma_start(out=tin_v(k, nj), in_=xp[b, :, jin : jin + nj, :]).then_inc(
            fam[k % NBUF], 16
        )

    def emit_double_store(eng, m):
        b, c = task(2 * m)
        jo = 2 * c
        eng.wait_ge(s_add, 2 * m + 2)
        eng.dma_start(out=op[b, :, jo : jo + 4, :], in_=tob_dv(m)).then_inc(
            s_store[m % 2], 16
        )

    def emit_single_store(eng, k):
        b, c = task(k)
        jo = 2 * c
        eng.wait_ge(s_add, k + 1)
        eng.dma_start(out=op[b, :, jo : jo + 2, :], in_=tob_task(k)).then_inc(
            s_store[(k // 2) % 2], 16
        )

    with nc.Block() as block:

        @block.vector
        def _(vector):
            for k in range(NG):
                sem_l, tgt_l = load_wait(k)
                vector.wait_ge(sem_l, tgt_l)
                if k >= 4:
                    # tob slot k%4 freed by double-store (k-4)//2 (same group)
                    mprev = (k - 4) // 2
                    vector.wait_ge(s_store[mprev % 2], 16 * (mprev // 2 + 1))
                in0, in1 = tin_pairs(k)
                vector.tensor_add(out=tob_task(k), in0=in0, in1=in1).then_inc(
                    s_add, 1
                )

        @block.sync
        def _(sync):
            for k in range(NG):
                if load_lane(k) == "sp":
                    emit_load(sync, k)
                if k >= 5 and k % 2 == 1:
                    m = (k - 5) // 2
                    if m <= NM - 3 and store_lane(m) == "sp":
                        emit_double_store(sync, m)
            # final two doubles split into singles for a short drain
            emit_single_store(sync, NG - 4)
            emit_single_store(sync, NG - 2)
            for g in range(2):
                # doubles 0..NM-3 by group, plus 2 singles per group
                tgt = 16 * len([m for m in range(NM - 2) if m % 2 == g])
                tgt += 16 * 2
                sync.wait_ge(s_store[g], tgt)

        @block.scalar
        def _(scalar):
            for k in range(NG):
                if load_lane(k) == "act":
                    emit_load(scalar, k)
                if k >= 5 and k % 2 == 1:
                    m = (k - 5) // 2
                    if m <= NM - 3 and store_lane(m) == "act":
                        emit_double_store(scalar, m)
            emit_single_store(scalar, NG - 3)
            emit_single_store(scalar, NG - 1)

        @block.gpsimd
        def _(gp):
            for k in range(NG):
                if load_lane(k) == "pool":
                    emit_load(gp, k)

    return nc


_CACHE = {}


def get_nc(repeat: int = 1) -> bass.Bass:
    key = f"nc{repeat}"
    if key not in _CACHE:
        _CACHE[key] = build(repeat)
    return _CACHE[key]


def finish_host(out_dev: np.ndarray) -> np.ndarray:
    """Upcast the device's bf16 sums and apply the exact *0.5."""
    out = np.asarray(out_dev).astype(np.float32)  # owned copy (dtype change)
    out *= np.float32(0.5)
    return out


def kernel(x: np.ndarray, **run_kwargs):
    from concourse.bass_utils import run_bass_kernel_spmd

    x = np.ascontiguousarray(np.asarray(x, dtype=np.float32))
    assert x.shape == (N_CORES * B_SHARD, J_IN, C, T), x.shape

    nc = get_nc()
    in_maps = [
        {"x": np.ascontiguousarray(x[i * B_SHARD : (i + 1) * B_SHARD])}
        for i in range(N_CORES)
    ]
    res = run_bass_kernel_spmd(nc, in_maps, core_ids=list(range(N_CORES)), **run_kwargs)
    out = np.concatenate(
        [finish_host(res.results[i]["out"]) for i in range(N_CORES)], axis=0
    )
    _CACHE["last_results"] = res
    return out


if __name__ == "__main__":
    from concourse.bass_interp import CoreSim

    nc = build(1)
    print("build ok")
    rng = np.random.default_rng(0)
    xx = rng.standard_normal((B_SHARD, J_IN, C, T)).astype(np.float32)
    sim = CoreSim(nc, trace=False)
    sim.tensor("x")[:] = xx
    sim.simulate(check_with_hw=False)
    print("CoreSim time:", sim.time, "ns")
    got = np.asarray(sim.tensor("out")).astype(np.float64) * 0.5
    IDX0 = np.array([0] + [2 * i - 1 for i in range(1, 16)])
    IDX1 = np.array([0] + [2 * i for i in range(1, 16)])
    exp = (xx[:, IDX0].astype(np.float64) + xx[:, IDX1].astype(np.float64)) * 0.5
    denom = np.maximum(np.abs(exp), 1e-6)
    err = np.max(np.abs(got - exp) / denom)
    print(f"rel err: {err:.3e}")
